# revision 16
# baseline (speedup 1.0000x reference)
"""Trainium2 Bass kernel for nn_CombinedAMLModel (dense_mlp, 8 NeuronCores).

Sharding: tensor-parallel over genes (20000 -> 2500/core), software-pipelined
over 2 sample chunks of 512 so phase B(chunk q) on the TensorEngine overlaps
phase A(chunk q+1) elementwise work.

Phase A per (125-gene tile, chunk): 12 relu-affine passes split by engine:
  - 'pe' (k in PE_KS): ACT a=relu(scl*x+bia) -> PE diag(coe) matmul into PSUM
  - 'dv' (k in DV_KS): DVE affine (2x mode), DVE fused relu*coe (max,mult),
        DVE tensor_tensor add into SBUF acc_d (all fp16)
  - 'dp' (k in DP_KS): DVE affine + fused relu*coe -> PE identity-diag matmul
        into the same PSUM group (keeps the PE HAM clock warm)
  Finalize (DVE): z = (psum + cst) + acc_d, fp16.
Phase B per chunk: out1[2000,512] = CW0 @ z (fp16 matmuls, K=2500 local),
  PSUM->SBUF copies alternating ACT/DVE per nt-pair (single engine per o
  tile), stores + two half ReduceScatters (fp16, rows 0-1000 / 1000-2000)
  per chunk, pipelined behind the stores. Tail per chunk: each core owns 125
  rows of each half after RS -> relu -> 2000->200 partial -> fp16 AllReduce
  (200x512) -> 200->20->1 replicated. Collectives trigger on gpsimd (which
  does no compute), DMA triggers split sync/gpsimd, diag matrices are
  host-precomputed and DMA'd once.
"""
import os
import sys

sys.path.insert(0, "/opt/trn_rl_repo")

import ml_dtypes
import numpy as np
from contextlib import ExitStack

import concourse.bass as bass
import concourse.tile as tile
from concourse import bacc, mybir
from concourse.bass_utils import run_bass_kernel_spmd

T, S, G, H = 3, 1024, 20000, 4
NCORES = 8
GL = G // NCORES
PT = 125
NGT = GL // PT
NK = T * H
N1, N2, N3 = 2000, 200, 20
PN = 125
NNT = N1 // PN
Q = 2
SC = S // Q

PE_KS = [0, 2, 4, 6, 8, 9, 11]  # ACT relu + PE diag(coe) macc
DV_KS = [1, 3, 5]               # full-DVE pass, add into acc_d
DP_KS = [7, 10]                 # DVE affine+relu*coe -> PE ident-diag add
ND = len(PE_KS) + 1             # diag slots: 7 coe + 1 identity

f32 = mybir.dt.float32
f16 = mybir.dt.float16
bf16 = mybir.dt.bfloat16

LAST_RUN = {}
_CACHE = {}


def _build_program():
    nc = bacc.Bacc("TRN2", target_bir_lowering=False, debug=False,
                   num_devices=NCORES)
    d = {}

    def inp(name, shape, dt=f32):
        d[name] = nc.dram_tensor(name, list(shape), dt, kind="ExternalInput").ap()

    inp("xT", (NGT, PT, T, S), f16)
    inp("scl", (PT, NGT * NK))
    inp("bia", (PT, NGT * NK))
    inp("coe", (PT, NGT * NK))
    inp("cst", (PT, NGT))
    inp("diags", (NGT, PT, ND * PT), f16)
    inp("cw0t", (NNT, PT, NGT * PN), f16)
    inp("cb0l", (PN, 2))
    inp("cw1l", (PN, 2 * N2), f16)
    inp("cb1", (100, 2))
    inp("cw2t", (100, 2 * N3), f16)
    inp("cb2", (N3, 1))
    inp("cwft", (N3, 1), f16)
    inp("cbf", (1, 1))
    out_d = nc.dram_tensor("out", [1, S], f32, kind="ExternalOutput").ap()

    Relu = mybir.ActivationFunctionType.Relu
    Ident = mybir.ActivationFunctionType.Identity
    MUL = mybir.AluOpType.mult
    ADD = mybir.AluOpType.add
    MAX = mybir.AluOpType.max

    with tile.TileContext(nc) as tc, ExitStack() as ctx:
        const = ctx.enter_context(tc.tile_pool(name="const", bufs=1))
        xpool = ctx.enter_context(tc.tile_pool(name="x", bufs=8))
        dpool = ctx.enter_context(tc.tile_pool(name="diag", bufs=NGT))
        apool = ctx.enter_context(tc.tile_pool(name="a", bufs=6))
        tpool = ctx.enter_context(tc.tile_pool(name="tmp", bufs=4))
        accp = ctx.enter_context(tc.tile_pool(name="acc", bufs=3))
        zpool = ctx.enter_context(tc.tile_pool(name="z", bufs=24))
        wpool = ctx.enter_context(tc.tile_pool(name="w0", bufs=NNT))
        opool = ctx.enter_context(tc.tile_pool(name="o1", bufs=3))
        ttpool = ctx.enter_context(tc.tile_pool(name="tail", bufs=2))
        zps = ctx.enter_context(tc.tile_pool(name="zps", bufs=2, space="PSUM"))
        mmps = ctx.enter_context(tc.tile_pool(name="mmps", bufs=4, space="PSUM"))
        tps = ctx.enter_context(tc.tile_pool(name="tps", bufs=2, space="PSUM"))
        dram = ctx.enter_context(tc.tile_pool(name="dram", bufs=1, space="DRAM"))

        # ---- constants ----
        CH = NGT * NK // 4
        sclt = const.tile([PT, NGT * NK], f32)
        biat = const.tile([PT, NGT * NK], f32)
        coet = const.tile([PT, NGT * NK], f32)
        for c4 in range(4):
            cs = slice(c4 * CH, (c4 + 1) * CH)
            nc.scalar.dma_start(sclt[:, cs], d["scl"][:, cs])
            nc.sync.dma_start(biat[:, cs], d["bia"][:, cs])
            nc.gpsimd.dma_start(coet[:, cs], d["coe"][:, cs])
        cstt = const.tile([PT, NGT], f32)
        nc.sync.dma_start(cstt[:], d["cst"][:])
        cb0t = const.tile([PN, 2], f32)
        nc.scalar.dma_start(cb0t[:], d["cb0l"][:])
        w1t = const.tile([PN, 2 * N2], f16)
        nc.scalar.dma_start(w1t[:], d["cw1l"][:])
        cb1t = const.tile([100, 2], f32)
        nc.scalar.dma_start(cb1t[:], d["cb1"][:])
        cw2tt = const.tile([100, 2 * N3], f16)
        nc.scalar.dma_start(cw2tt[:], d["cw2t"][:])
        cb2t = const.tile([N3, 1], f32)
        nc.scalar.dma_start(cb2t[:], d["cb2"][:])
        cwftt = const.tile([N3, 1], f16)
        nc.scalar.dma_start(cwftt[:], d["cwft"][:])
        cbft = const.tile([1, 1], f32)
        nc.scalar.dma_start(cbft[:], d["cbf"][:])

        # ---- DRAM scratch for collectives ----
        partial_h = [[dram.tile([N1 // 2, SC], f16, tag=f"pc{q}h{h}",
                                name=f"partial{q}h{h}") for h in range(2)]
                     for q in range(Q)]
        rs_out = [[dram.tile([PN, SC], f16, tag=f"rs{q}h{h}",
                             name=f"rsout{q}h{h}") for h in range(2)]
                  for q in range(Q)]
        partial2 = [dram.tile([N2, SC], f16, tag=f"p2{q}", name=f"partial2{q}")
                    for q in range(Q)]
        summed2 = [dram.tile([N2, SC], f16, tag=f"s2{q}", name=f"summed2{q}")
                   for q in range(Q)]
        ccwarm_in = dram.tile([1, 128], f32, tag="ccwi")
        ccwarm_out = dram.tile([1, 128], f32, tag="ccwo")

        ccwarm_sb = const.tile([1, 128], f32)
        nc.gpsimd.memset(ccwarm_sb[:], 0.0)
        nc.gpsimd.dma_start(ccwarm_in[:], ccwarm_sb[:])
        nc.gpsimd.collective_compute(
            "AllReduce", mybir.AluOpType.add,
            replica_groups=[list(range(NCORES))],
            ins=[ccwarm_in.opt()], outs=[ccwarm_out.opt()],
        )

        z_tiles = {}

        def load_w0(nt):
            w = wpool.tile([PT, NGT * PN], f16, tag="w0", name=f"w0_{nt}")
            nc.gpsimd.dma_start(w[:], d["cw0t"][nt, :, :])
            return w

        x_tiles = {}

        def issue_x(q, g0=0, g1=NGT, force_gp=False):
            for gt in range(g0, g1):
                xt = xpool.tile([PT, T, SC], f16, tag="x", name=f"x{q}_{gt}")
                eng = nc.gpsimd if force_gp else (nc.sync, nc.gpsimd)[gt % 2]
                for t in range(T):
                    eng.dma_start(xt[:, t, :],
                                  d["xT"][gt, :, t, q * SC:(q + 1) * SC])
                x_tiles[(q, gt)] = xt

        dg_tiles = {}

        def phase_a_tile(q, gt):
            if True:
                xt = x_tiles[(q, gt)]
                dg = dg_tiles[gt]
                ps = zps.tile([PT, SC], f32, tag="zps", name=f"zps{q}_{gt}")
                acc_d = accp.tile([PT, SC], f16, tag="accd")
                first_d = True
                nmm = 0
                for k in range(NK):
                    t = k // H
                    ci = gt * NK + k
                    xin = xt[:, t, :]
                    if k in PE_KS:
                        a = apool.tile([PT, SC], f16, tag="a")
                        nc.scalar.activation(a[:], xin, Relu,
                                             bias=biat[:, ci:ci + 1],
                                             scale=sclt[:, ci:ci + 1])
                        di = PE_KS.index(k)
                        nc.tensor.matmul(ps[:], dg[:, di * PT:(di + 1) * PT],
                                         a[:], start=(nmm == 0),
                                         stop=(nmm == len(PE_KS) + len(DP_KS) - 1))
                        nmm += 1
                    else:
                        v = apool.tile([PT, SC], f16, tag="a")
                        nc.vector.tensor_scalar(v[:], xin,
                                                sclt[:, ci:ci + 1],
                                                biat[:, ci:ci + 1], MUL, ADD)
                        if k in DP_KS:
                            tmp = apool.tile([PT, SC], f16, tag="a")
                            nc.vector.tensor_scalar(tmp[:], v[:], 0.0,
                                                    coet[:, ci:ci + 1],
                                                    MAX, MUL)
                            nc.tensor.matmul(ps[:],
                                             dg[:, ND * PT - PT:ND * PT],
                                             tmp[:], start=(nmm == 0),
                                             stop=False)
                            nmm += 1
                        elif first_d:
                            nc.vector.tensor_scalar(acc_d[:], v[:], 0.0,
                                                    coet[:, ci:ci + 1],
                                                    MAX, MUL)
                            first_d = False
                        else:
                            tmp = tpool.tile([PT, SC], f16, tag="tmp")
                            nc.vector.tensor_scalar(tmp[:], v[:], 0.0,
                                                    coet[:, ci:ci + 1],
                                                    MAX, MUL)
                            nc.vector.tensor_tensor(acc_d[:], acc_d[:],
                                                    tmp[:], ADD)
                # finalize: z = (psum + cst) + acc_d
                zc = tpool.tile([PT, SC], f16, tag="tmp", name=f"zc{q}_{gt}")
                nc.vector.tensor_scalar(zc[:], ps[:], 1.0, cstt[:, gt:gt + 1],
                                        MUL, ADD)
                z = zpool.tile([PT, SC], f16, tag="z", name=f"z{q}_{gt}")
                nc.vector.tensor_tensor(z[:], zc[:], acc_d[:], ADD)
                z_tiles[(q, gt)] = z

        def phase_b_group(q, ng):
            h = ng // 4
            partial_r = partial_h[q][h][:].rearrange("(n p) s -> p n s", p=PN)
            ng2 = ng % 4
            o = opool.tile([PN, 2 * SC], f16, tag="o1")
            for j in range(2):
                nt = ng * 2 + j
                w = w_tiles[nt]
                pp = mmps.tile([PN, SC], f32, tag="mm", name=f"mm{q}_{nt}")
                for gt in range(NGT):
                    nc.tensor.matmul(pp[:], w[:, gt * PN:(gt + 1) * PN],
                                     z_tiles[(q, gt)][:],
                                     start=(gt == 0), stop=(gt == NGT - 1))
                if ng % 2 == 0:
                    nc.scalar.copy(o[:, j * SC:(j + 1) * SC], pp[:])
                else:
                    nc.vector.tensor_scalar(o[:, j * SC:(j + 1) * SC],
                                            pp[:], 1.0, None, MUL)
            nc.sync.dma_start(partial_r[:, 2 * ng2:2 * ng2 + 2, :], o[:])
            if ng2 == 3:
                nc.gpsimd.collective_compute(
                    "ReduceScatter", mybir.AluOpType.add,
                    replica_groups=[list(range(NCORES))],
                    ins=[partial_h[q][h].opt()], outs=[rs_out[q][h].opt()],
                )

        z1s = {}

        def pre_tail_h(q, j):
            y1 = ttpool.tile([PN, SC], f16, tag="y1", name=f"y1_{q}{j}")
            nc.sync.dma_start(y1[:], rs_out[q][j][:])
            z1j = ttpool.tile([PN, SC], f16, tag="z1", name=f"z1_{q}{j}")
            nc.scalar.activation(z1j[:], y1[:], Relu,
                                 bias=cb0t[:, j:j + 1], scale=1.0)
            z1s[(q, j)] = z1j

        def pre_tail(q):
            z1 = [z1s[(q, 0)], z1s[(q, 1)]]
            for mc in range(2):
                p2 = tps.tile([100, SC], f32, tag="tps", name=f"p2_{q}{mc}")
                for j in range(2):
                    nc.tensor.matmul(
                        p2[:],
                        w1t[:, j * N2 + mc * 100:j * N2 + (mc + 1) * 100],
                        z1[j][:], start=(j == 0), stop=(j == 1))
                o2 = ttpool.tile([100, SC], f16, tag="o2", name=f"o2_{q}{mc}")
                nc.scalar.copy(o2[:], p2[:])
                nc.sync.dma_start(partial2[q][mc * 100:(mc + 1) * 100, :],
                                  o2[:])
            nc.gpsimd.collective_compute(
                "AllReduce", mybir.AluOpType.add,
                replica_groups=[list(range(NCORES))],
                ins=[partial2[q].opt()], outs=[summed2[q].opt()],
            )

        def post_tail(q):
            z2 = ttpool.tile([100, 2 * SC], f16, tag="z2", name=f"z2_{q}")
            for mc in range(2):
                y2 = ttpool.tile([100, SC], f16, tag="y2", name=f"y2_{q}{mc}")
                nc.sync.dma_start(y2[:], summed2[q][mc * 100:(mc + 1) * 100, :])
                nc.scalar.activation(z2[:, mc * SC:(mc + 1) * SC], y2[:], Relu,
                                     bias=cb1t[:, mc:mc + 1], scale=1.0)
            p3 = tps.tile([N3, SC], f32, tag="tps", name=f"p3_{q}")
            for mc in range(2):
                nc.tensor.matmul(p3[:], cw2tt[:, mc * N3:(mc + 1) * N3],
                                 z2[:, mc * SC:(mc + 1) * SC],
                                 start=(mc == 0), stop=(mc == 1))
            z3 = ttpool.tile([N3, SC], f16, tag="z3", name=f"z3_{q}")
            nc.scalar.activation(z3[:], p3[:], Relu, bias=cb2t[:], scale=1.0)
            p4 = tps.tile([1, SC], f32, tag="tps", name=f"p4_{q}")
            nc.tensor.matmul(p4[:], cwftt[:], z3[:], start=True, stop=True)
            outc = ttpool.tile([1, SC], f32, tag="outc", name=f"outc_{q}")
            nc.scalar.activation(outc[:], p4[:], Ident, bias=cbft[:],
                                 scale=1.0)
            nc.sync.dma_start(out_d[:, q * SC:(q + 1) * SC], outc[:])

        for gt in range(NGT):
            issue_x(0, gt, gt + 1)
            xt0 = x_tiles[(0, gt)]
            dgt = dpool.tile([PT, ND * PT], f16, tag="diag", name=f"dg{gt}")
            nc.gpsimd.dma_start(dgt[:], d["diags"][gt, :, :])
            dg_tiles[gt] = dgt
        w_tiles = [load_w0(nt) for nt in range(NNT)]
        for gt in range(NGT):
            phase_a_tile(0, gt)
        issue_x(1, 0, 8)
        ai = 0
        for ng in range(NNT // 2):
            phase_b_group(0, ng)
            if ng == 0:
                issue_x(1, 8, 14, force_gp=True)
            elif ng == 1:
                issue_x(1, 14, NGT, force_gp=True)
            while ai < (ng + 1) * NGT // (NNT // 2):
                phase_a_tile(1, ai)
                ai += 1
        for ng in range(NNT // 2):
            phase_b_group(1, ng)
            if ng == 0:
                pre_tail_h(0, 0)
            elif ng == 2:
                pre_tail_h(0, 1)
            elif ng == 3:
                pre_tail(0)
            elif ng == 5:
                post_tail(0)
            elif ng == 6:
                pre_tail_h(1, 0)
        pre_tail_h(1, 1)
        pre_tail(1)
        post_tail(1)

    nc.compile()
    return nc


def _shard_inputs(x, W1, b1, W2, b2, Wc, bc,
                  CW0, Cb0, CW1, Cb1, CW2, Cb2, CWf, Cbf):
    f = lambda a: np.ascontiguousarray(a, dtype=np.float32)
    h16 = lambda a: np.ascontiguousarray(a).astype(np.float16)
    CW1T = np.ascontiguousarray(CW1.T)
    CW2T = np.ascontiguousarray(CW2.T)
    shared = {
        "cb1": f(Cb1.reshape(2, 100).T),
        "cw2t": h16(CW2T.reshape(2, 100, N3).transpose(1, 0, 2)
                    .reshape(100, 2 * N3)),
        "cb2": f(Cb2.reshape(N3, 1)),
        "cwft": h16(CWf.T),
        "cbf": f(Cbf.reshape(1, 1)),
    }
    in_maps = []
    for c in range(NCORES):
        gs = slice(c * GL, (c + 1) * GL)
        scl = W1[:, gs, :].transpose(1, 0, 2).reshape(GL, NK)
        bia = b1[:, gs, :].transpose(1, 0, 2).reshape(GL, NK)
        coe = (W2[:, gs, :] * Wc[gs, :].T[:, :, None]) \
            .transpose(1, 0, 2).reshape(GL, NK)
        cst = (b2[:, gs] * Wc[gs, :].T).sum(0) + bc[gs]
        # diag tiles: (NGT, PT, ND*PT) fp16: 7 coe diags + 1 identity
        dvals = np.concatenate(
            [coe[:, PE_KS], np.ones((GL, 1), np.float32)], axis=1)
        diags = np.zeros((NGT, PT, ND, PT), dtype=np.float32)
        dv = dvals.reshape(NGT, PT, ND)
        idx = np.arange(PT)
        diags[:, idx, :, idx] = dv[:, idx, :].transpose(1, 0, 2)
        diags = diags.reshape(NGT, PT, ND * PT)
        h0 = c * PN
        h1 = N1 // 2 + c * PN
        in_maps.append({
            "xT": np.ascontiguousarray(
                x[:, :, gs].transpose(2, 0, 1).reshape(NGT, PT, T, S)
            ).astype(np.float16),
            "scl": f(scl.reshape(NGT, PT, NK).transpose(1, 0, 2)
                     .reshape(PT, NGT * NK)),
            "bia": f(bia.reshape(NGT, PT, NK).transpose(1, 0, 2)
                     .reshape(PT, NGT * NK)),
            "coe": f(coe.reshape(NGT, PT, NK).transpose(1, 0, 2)
                     .reshape(PT, NGT * NK)),
            "cst": f(cst.reshape(NGT, PT).T),
            "diags": diags.astype(np.float16),
            "cw0t": h16(CW0[:, gs].reshape(NNT, PN, NGT, PT)
                        .transpose(0, 3, 2, 1).reshape(NNT, PT, NGT * PN)),
            "cb0l": f(np.stack([Cb0[h0:h0 + PN], Cb0[h1:h1 + PN]], axis=1)),
            "cw1l": h16(np.stack([CW1T[h0:h0 + PN, :], CW1T[h1:h1 + PN, :]],
                                 axis=1).reshape(PN, 2 * N2)),
            **shared,
        })
    return in_maps


def _install_profile_shim():
    import types
    try:
        import antenv.axon_hooks  # noqa: F401
        return True
    except ImportError:
        pass
    try:
        import antenv
        from trn_agent_boot.trn_boot import _ntff_profile_via_ctypes
        hook = _ntff_profile_via_ctypes("/opt/axon/libaxon_pjrt.so")
        mod = types.ModuleType("antenv.axon_hooks")
        mod.get_axon_ntff_profile_hook = lambda: hook
        mod.set_axon_ntff_profile_hook = lambda h: None
        sys.modules["antenv.axon_hooks"] = mod
        antenv.axon_hooks = mod
        return hook is not None
    except Exception:
        return False


def kernel(**inputs):
    inputs = {k: np.asarray(v) for k, v in inputs.items()}
    in_maps = _shard_inputs(**inputs)
    if "nc" not in _CACHE:
        _CACHE["nc"] = _build_program()
    nc = _CACHE["nc"]
    trace = bool(os.environ.get("KERNEL_PROFILE")) and _install_profile_shim()
    res = run_bass_kernel_spmd(nc, in_maps, core_ids=list(range(NCORES)),
                               trace=trace)
    LAST_RUN["exec_time_ns"] = res.exec_time_ns
    LAST_RUN["mean_exec_time_ns"] = res.mean_exec_time_ns
    if res.instructions_and_trace is not None:
        LAST_RUN["trace_path"] = res.instructions_and_trace[1]
    return res.results[0]["out"].reshape(1, S, 1)


if __name__ == "__main__":
    rng = np.random.default_rng(0)
    ins = {
        "x": rng.standard_normal((T, S, G), dtype=np.float32),
        "W1": rng.standard_normal((T, G, H), dtype=np.float32) * 0.5,
        "b1": rng.standard_normal((T, G, H), dtype=np.float32) * 0.1,
        "W2": rng.standard_normal((T, G, H), dtype=np.float32) * 0.5,
        "b2": rng.standard_normal((T, G), dtype=np.float32) * 0.1,
        "Wc": rng.standard_normal((G, T), dtype=np.float32) * 0.5,
        "bc": rng.standard_normal((G,), dtype=np.float32) * 0.1,
        "CW0": rng.standard_normal((N1, G), dtype=np.float32) * 0.007,
        "Cb0": rng.standard_normal((N1,), dtype=np.float32) * 0.007,
        "CW1": rng.standard_normal((N2, N1), dtype=np.float32) * 0.02,
        "Cb1": rng.standard_normal((N2,), dtype=np.float32) * 0.02,
        "CW2": rng.standard_normal((N3, N2), dtype=np.float32) * 0.07,
        "Cb2": rng.standard_normal((N3,), dtype=np.float32) * 0.07,
        "CWf": rng.standard_normal((1, N3), dtype=np.float32) * 0.2,
        "Cbf": rng.standard_normal((1,), dtype=np.float32) * 0.2,
    }
    out = kernel(**ins)
    xx = ins["x"]
    h = np.maximum(xx[..., None] * ins["W1"][:, None] + ins["b1"][:, None], 0.0)
    y = np.einsum("tsgh,tgh->tsg", h, ins["W2"]) + ins["b2"][:, None, :]
    zz = np.einsum("tsg,gt->sg", y, ins["Wc"]) + ins["bc"]
    for Wl, bl in ((ins["CW0"], ins["Cb0"]), (ins["CW1"], ins["Cb1"]),
                   (ins["CW2"], ins["Cb2"])):
        zz = np.maximum(zz @ Wl.T + bl, 0.0)
    ref = (zz @ ins["CWf"].T + ins["Cbf"])[None]
    err = np.abs(out - ref).max() / (np.abs(ref).max() + 1e-12)
    print("self-test rel err:", err)
    print("exec_time_ns:", LAST_RUN.get("exec_time_ns"))
